# revision 1
# baseline (speedup 1.0000x reference)
"""Trainium2 Bass kernel for nn_AttentionHAN (histogram_binning).

Strategy
--------
The reference network collapses algebraically:
  - t_K is dead; t_Q/i_Q/i_K and the output projection fold into small
    input-space matrices (computed on host from the replicated params).
  - Per batch row the device only needs 13 values:
      sp(4)  = pre-sigmoid attention scores
      tvd(4) = per-head dot of t_V with Wout[0,:128]
      ivd(4) = per-head dot of i_V with Wout[0,:128]
      base(1)= contribution of [t_Q, i_Q] @ Wout[0,128:] + bout
    plus the chi-square statistics of t_V/i_V, which reduce to per-feature
    counts S = #(v > thr) and C = #(v > thr and label==1).
  - out[b] = base + sum_h [ at*m1 + ai*m2 - (at*ai)*m3 ],
      m1 = s*tvd, m2 = s*ivd, m3 = s*m2,  s = sigmoid(sp),
    where at/ai = alpha_t/alpha_i depend on the GLOBAL chi statistics.

Sharding: pure data parallel over B on 8 cores (16384 rows each).  The tiny
per-core (128,) count tables are reduced on host (the "all-reduce" of the
sharding hint), alpha is computed exactly as the reference does, and a second
small kernel applies the 13-coefficient combination per row.

Launch A (per core, feature-on-partition, fp32r matmuls):
  tv.T(128f,512b)/iv.T accumulated over K=256 in PSUM; one DVE tensor_scalar
  (is_gt, per-partition threshold, accum_out) both binarizes and emits the
  per-block S count column; a K=1 PE matmul broadcasts the label row across
  partitions and tensor_tensor_reduce emits the per-block C column; the
  sm.T(13,512) matmul + ACT Identity(+bias) emits the 13-row R tensor.
Launch B (per core): PE-transposes R blocks to batch-on-partition, applies
  sigmoid + the coefficient combination.

All matmul operands use float32r (tf32 input rounding, exact products, fp32
accumulation); the host pre-rounds inputs so device numerics are
deterministic.  End-to-end error vs the fp32 reference is ~3e-4.
"""

import sys
import numpy as np

sys.path.insert(0, "/opt/trn_rl_repo")

import concourse.bacc as bacc  # noqa: E402
import concourse.tile as tile  # noqa: E402
from concourse import mybir  # noqa: E402

F32 = mybir.dt.float32
F32R = mybir.dt.float32r
f32 = np.float32


def _tf32(a):
    """Round-to-nearest-even to the tf32 grid (fp32r input quantization)."""
    u = np.ascontiguousarray(a, dtype=np.float32).view(np.uint32)
    add = np.uint32(0x00001000) + ((u >> np.uint32(13)) & np.uint32(1))
    return ((u + add) & np.uint32(0xFFFFE000)).view(np.float32)


B_TOT = 131072
IN = 256
HID = 128
H = 4
D = 32
NCORES = 8
THRESH = 0.7
BLK = 512
RPC = B_TOT // NCORES          # 16384 rows per core
NBLK = RPC // BLK              # 32 blocks of 512
SUPER = [2048] * 7 + [1024, 512, 512]  # kernel A superblocks (sum = RPC)
SUPER_B = [4096] * 4           # kernel B superblock/group sizes (sum = RPC)
XBUFS = 3                      # kernel A x-tile buffering depth

_cache = {}


def _build_kernel_a():
    nc = bacc.Bacc("TRN2", target_bir_lowering=False, debug=False)
    xt = nc.dram_tensor("xt", (IN, RPC), F32R, kind="ExternalInput")
    xi = nc.dram_tensor("xi", (IN, RPC), F32R, kind="ExternalInput")
    lab = nc.dram_tensor("lab", (1, BLK), F32R, kind="ExternalInput")
    ones = nc.dram_tensor("ones", (1, 128), F32R, kind="ExternalInput")
    wtv = nc.dram_tensor("wtv", (IN, HID), F32R, kind="ExternalInput")
    wiv = nc.dram_tensor("wiv", (IN, HID), F32R, kind="ExternalInput")
    wsmt = nc.dram_tensor("wsmt", (IN, 13), F32R, kind="ExternalInput")
    wsmi = nc.dram_tensor("wsmi", (IN, 13), F32R, kind="ExternalInput")
    thrt = nc.dram_tensor("thrt", (HID, 1), F32, kind="ExternalInput")
    thri = nc.dram_tensor("thri", (HID, 1), F32, kind="ExternalInput")
    bsm = nc.dram_tensor("bsm", (13, 1), F32, kind="ExternalInput")
    idt = nc.dram_tensor("idt", (13, 13), F32, kind="ExternalInput")
    m_out = nc.dram_tensor("m_out", (128, 12 * NBLK * 4), F32,
                           kind="ExternalOutput")
    aux_out = nc.dram_tensor("aux_out", (128, 2 * NBLK + 2 + NBLK * 4), F32,
                             kind="ExternalOutput")

    sb_max = max(SUPER)
    with tile.TileContext(nc) as tc:
        with (
            tc.tile_pool(name="w", bufs=1) as wp,
            tc.tile_pool(name="x", bufs=XBUFS) as xp,
            tc.tile_pool(name="fv", bufs=3) as fp,
            tc.tile_pool(name="acc", bufs=1) as ap,
            tc.tile_pool(name="rout", bufs=3) as rp,
            tc.tile_pool(name="ptv", bufs=2, space="PSUM") as ptvp,
            tc.tile_pool(name="piv", bufs=2, space="PSUM") as pivp,
            tc.tile_pool(name="psm", bufs=2, space="PSUM") as psmp,
            tc.tile_pool(name="ptr", bufs=2, space="PSUM") as ptrp,
        ):
            wtv_sb = [wp.tile([128, HID], F32R, name=f"wtv{k}", tag=f"wtv{k}")
                      for k in range(2)]
            wiv_sb = [wp.tile([128, HID], F32R, name=f"wiv{k}", tag=f"wiv{k}")
                      for k in range(2)]
            wsmt_sb = [wp.tile([128, 13], F32R, name=f"wsmt{k}", tag=f"wsmt{k}")
                       for k in range(2)]
            wsmi_sb = [wp.tile([128, 13], F32R, name=f"wsmi{k}", tag=f"wsmi{k}")
                       for k in range(2)]
            for k in range(2):
                sl = slice(k * 128, (k + 1) * 128)
                nc.sync.dma_start(wtv_sb[k][:], wtv[sl, :])
                nc.sync.dma_start(wiv_sb[k][:], wiv[sl, :])
                nc.sync.dma_start(wsmt_sb[k][:], wsmt[sl, :])
                nc.sync.dma_start(wsmi_sb[k][:], wsmi[sl, :])
            thrt_sb = wp.tile([HID, 1], F32, tag="thrt")
            thri_sb = wp.tile([HID, 1], F32, tag="thri")
            bsm_sb = wp.tile([13, 1], F32, tag="bsm")
            ones_sb = wp.tile([1, 128], F32R, tag="ones")
            idt_sb = wp.tile([13, 13], F32, tag="idt")
            nc.sync.dma_start(thrt_sb[:], thrt[:])
            nc.sync.dma_start(thri_sb[:], thri[:])
            nc.sync.dma_start(bsm_sb[:], bsm[:])
            nc.sync.dma_start(ones_sb[:], ones[:])
            nc.sync.dma_start(idt_sb[:], idt[:])

            aux_sb = ap.tile([128, 2 * NBLK + 2 + NBLK * 4], F32, tag="aux")
            st_sb = aux_sb[:, 0:NBLK]
            si_sb = aux_sb[:, NBLK:2 * NBLK]
            ct_sb = aux_sb[:, 2 * NBLK:2 * NBLK + 1]
            ci_sb = aux_sb[:, 2 * NBLK + 1:2 * NBLK + 2]
            base_sb = aux_sb[:, 2 * NBLK + 2:]
            lab_sb = ap.tile([1, BLK], F32R, tag="lab")
            nc.sync.dma_start(lab_sb[:], lab[:])
            mt = ap.tile([128, 12 * NBLK * 4], F32, tag="mt")
            pending = []

            def emit_products(item):
                # deferred by one block so PE's transposes never make the
                # next block's matmuls wait on the ACT identity copy
                prt, po, pblk = item
                ptr = ptrp.tile([128, 52], F32, name="ptr", tag="ptr")
                for c in range(4):
                    nc.tensor.transpose(
                        ptr[:, c * 13:(c + 1) * 13],
                        prt[0:13, po + c * 128:po + (c + 1) * 128],
                        idt_sb[:])
                p3 = ptr[:].rearrange("p (g k) -> p g k", k=13)
                s = fp.tile([128, 16], F32, name="s", tag="s")
                s3 = s[:].rearrange("p (g k) -> p g k", k=4)
                nc.scalar.activation(
                    s3, p3[:, :, 0:4], mybir.ActivationFunctionType.Sigmoid)
                mbv = mt[:].rearrange("p (g k) -> p g k", k=12)
                mb3 = mbv[:, 4 * pblk:4 * pblk + 4, :]
                nc.vector.tensor_tensor(
                    mb3[:, :, 0:4], s3, p3[:, :, 4:8], op=mybir.AluOpType.mult)
                nc.vector.tensor_tensor(
                    mb3[:, :, 4:8], s3, p3[:, :, 8:12], op=mybir.AluOpType.mult)
                nc.vector.tensor_tensor(
                    mb3[:, :, 8:12], s3, mb3[:, :, 4:8], op=mybir.AluOpType.mult)
                nc.vector.tensor_copy(
                    base_sb[:, pblk * 4:(pblk + 1) * 4], p3[:, :, 12])

            def emit_and_flush(item):
                emit_products(item)
                pblk = item[2]
                if (pblk + 1) % 4 == 0:  # superblock of M complete -> stream out
                    c0 = (pblk - 3) * 4 * 12
                    c1 = (pblk + 1) * 4 * 12
                    nc.sync.dma_start(m_out[:, c0:c1], mt[:, c0:c1])

            blk = 0
            off = 0
            for size in SUPER:
                xt0 = xp.tile([128, sb_max], F32R, tag="xt0")
                xt1 = xp.tile([128, sb_max], F32R, tag="xt1")
                xi0 = xp.tile([128, sb_max], F32R, tag="xi0")
                xi1 = xp.tile([128, sb_max], F32R, tag="xi1")
                nc.sync.dma_start(xt0[:, :size], xt[0:128, off:off + size])
                nc.sync.dma_start(xt1[:, :size], xt[128:256, off:off + size])
                nc.sync.dma_start(xi0[:, :size], xi[0:128, off:off + size])
                nc.sync.dma_start(xi1[:, :size], xi[128:256, off:off + size])
                rt = rp.tile([13, sb_max], F32, tag="rt")
                for j in range(size // BLK):
                    o = j * BLK
                    ptv = ptvp.tile([128, BLK], F32)
                    piv = pivp.tile([128, BLK], F32)
                    psm = psmp.tile([13, BLK], F32)
                    nc.tensor.matmul(ptv[:], wtv_sb[0][:], xt0[:, o:o + BLK],
                                     start=True, stop=False)
                    nc.tensor.matmul(ptv[:], wtv_sb[1][:], xt1[:, o:o + BLK],
                                     start=False, stop=True)
                    nc.tensor.matmul(piv[:], wiv_sb[0][:], xi0[:, o:o + BLK],
                                     start=True, stop=False)
                    nc.tensor.matmul(piv[:], wiv_sb[1][:], xi1[:, o:o + BLK],
                                     start=False, stop=True)
                    nc.tensor.matmul(psm[:], wsmt_sb[0][:], xt0[:, o:o + BLK],
                                     start=True, stop=False)
                    nc.tensor.matmul(psm[:], wsmt_sb[1][:], xt1[:, o:o + BLK],
                                     start=False, stop=False)
                    nc.tensor.matmul(psm[:], wsmi_sb[0][:], xi0[:, o:o + BLK],
                                     start=False, stop=False)
                    nc.tensor.matmul(psm[:], wsmi_sb[1][:], xi1[:, o:o + BLK],
                                     start=False, stop=True)
                    fvt = fp.tile([128, BLK], F32, tag="fvt")
                    fvi = fp.tile([128, BLK], F32, tag="fvi")
                    # binarize + S count in one op
                    nc.vector.tensor_scalar(
                        fvt[:], ptv[:], thrt_sb[:], None,
                        op0=mybir.AluOpType.is_gt, op1=mybir.AluOpType.add,
                        accum_out=st_sb[:, blk:blk + 1])
                    nc.vector.tensor_scalar(
                        fvi[:], piv[:], thri_sb[:], None,
                        op0=mybir.AluOpType.is_gt, op1=mybir.AluOpType.add,
                        accum_out=si_sb[:, blk:blk + 1])
                    if blk == NBLK - 1:
                        # the single possibly-mixed block: per-feature count
                        # of (v > thr) rows with label==1.  Label row is
                        # broadcast across partitions via a K=1 matmul; one
                        # PSUM operand max per DVE op -> fv from SBUF.
                        plab = ptrp.tile([128, BLK], F32, name="plab",
                                         tag="ptr")
                        nc.tensor.matmul(plab[:], ones_sb[:], lab_sb[:],
                                         start=True, stop=True)
                        fvl = fp.tile([128, BLK], F32, tag="fvl")
                        nc.vector.scalar_tensor_tensor(
                            fvl[:], fvt[:], 1.0, plab[:],
                            op0=mybir.AluOpType.mult, op1=mybir.AluOpType.mult,
                            accum_out=ct_sb[:, 0:1])
                        nc.vector.scalar_tensor_tensor(
                            fvl[:], fvi[:], 1.0, plab[:],
                            op0=mybir.AluOpType.mult, op1=mybir.AluOpType.mult,
                            accum_out=ci_sb[:, 0:1])
                    nc.scalar.activation(
                        rt[:, o:o + BLK], psm[:],
                        mybir.ActivationFunctionType.Identity, bias=bsm_sb[:])
                    pending.append((rt, o, blk))
                    if len(pending) > 1:
                        emit_and_flush(pending.pop(0))
                    blk += 1
                off += size
            while pending:
                emit_and_flush(pending.pop(0))
            nc.sync.dma_start(aux_out[:], aux_sb[:])

    nc.compile()
    return nc


def _build_kernel_b():
    nc = bacc.Bacc("TRN2", target_bir_lowering=False, debug=False)
    mb = nc.dram_tensor("mb", (128, 13 * NBLK * 4), F32, kind="ExternalInput")
    crep = nc.dram_tensor("crep", (128, 384), F32, kind="ExternalInput")
    o_out = nc.dram_tensor("o_out", (128, NBLK * 4), F32, kind="ExternalOutput")

    nchunk = NBLK * 4
    ngrp = 4
    cpg = nchunk // ngrp  # chunks per group
    with tile.TileContext(nc) as tc:
        with (
            tc.tile_pool(name="w", bufs=1) as wp,
            tc.tile_pool(name="m", bufs=3) as mp,
            tc.tile_pool(name="t", bufs=2) as tp,
            tc.tile_pool(name="out", bufs=1) as op,
        ):
            crep_sb = wp.tile([128, 384], F32, tag="crep")
            nc.sync.dma_start(crep_sb[:], crep[:])
            base_sb = wp.tile([128, nchunk], F32, tag="base")
            out_sb = op.tile([128, nchunk], F32, tag="o")

            mts, mcs = [], []
            for gi in range(ngrp):
                mtile = mp.tile([128, 12 * cpg], F32, name=f"mt{gi}", tag="mt")
                nc.sync.dma_start(
                    mtile[:], mb[:, gi * 12 * cpg:(gi + 1) * 12 * cpg])
                mts.append(mtile)
            nc.sync.dma_start(base_sb[:], mb[:, 12 * nchunk:])
            for gi in range(ngrp):
                mc = tp.tile([128, 12 * cpg], F32, name=f"mc{gi}", tag="mc")
                mcs.append(mc)
                nc.vector.tensor_tensor(
                    mc[:], mts[gi][:], crep_sb[:, 0:12 * cpg],
                    op=mybir.AluOpType.mult)
            reds = []
            for gi in range(ngrp):
                red = tp.tile([128, cpg], F32, name=f"red{gi}", tag="red")
                reds.append(red)
                nc.vector.tensor_reduce(
                    red[:], mcs[gi][:].rearrange("p (g k) -> p g k", k=12),
                    axis=mybir.AxisListType.X, op=mybir.AluOpType.add)
            for gi in range(ngrp):
                nc.vector.tensor_tensor(
                    out_sb[:, gi * cpg:(gi + 1) * cpg], reds[gi][:],
                    base_sb[:, gi * cpg:(gi + 1) * cpg],
                    op=mybir.AluOpType.add)
                nc.sync.dma_start(o_out[:, gi * cpg:(gi + 1) * cpg],
                                  out_sb[:, gi * cpg:(gi + 1) * cpg])

    nc.compile()
    return nc


def _get_kernels():
    if "a" not in _cache:
        _cache["a"] = _build_kernel_a()
        _cache["b"] = _build_kernel_b()
    return _cache["a"], _cache["b"]


class _Runner:
    """Persistent jitted SPMD executor for a compiled Bass module.

    Mirrors bass2jax.run_bass_via_pjrt but keeps the jitted callable alive so
    repeated kernel() invocations skip retracing/recompilation."""

    def __init__(self, nc):
        import jax
        from jax.sharding import Mesh, PartitionSpec
        from jax.experimental.shard_map import shard_map
        from concourse import bass2jax

        bass2jax.install_neuronx_cc_hook()
        self._nc = nc
        pname = nc.partition_id_tensor.name if nc.partition_id_tensor else None
        in_names, out_names, out_avals = [], [], []
        self._zero_outs = []
        for alloc in nc.m.functions[0].allocations:
            if not isinstance(alloc, mybir.MemoryLocationSet):
                continue
            nm = alloc.memorylocations[0].name
            if alloc.kind == "ExternalInput":
                if nm != pname:
                    in_names.append(nm)
            elif alloc.kind == "ExternalOutput":
                out_names.append(nm)
                shape = tuple(alloc.tensor_shape)
                dt = mybir.dt.np(alloc.dtype)
                out_avals.append(jax.core.ShapedArray(shape, dt))
                self._zero_outs.append(np.zeros(shape, dt))
        self._in_names = in_names
        self._out_names = out_names
        all_in_names = in_names + out_names + ([pname] if pname else [])

        def _body(*args):
            operands = list(args)
            if pname:
                operands.append(bass2jax.partition_id_tensor())
            outs = bass2jax._bass_exec_p.bind(
                *operands, out_avals=tuple(out_avals),
                in_names=tuple(all_in_names), out_names=tuple(out_names),
                lowering_input_output_aliases=(), sim_require_finite=True,
                sim_require_nnan=True, nc=nc)
            return tuple(outs)

        devices = jax.devices()[:NCORES]
        assert len(devices) == NCORES, f"need {NCORES} devices"
        mesh = Mesh(np.asarray(devices), ("core",))
        nio = len(in_names) + len(out_names)
        self._fn = jax.jit(
            shard_map(_body, mesh=mesh,
                      in_specs=(PartitionSpec("core"),) * nio,
                      out_specs=(PartitionSpec("core"),) * len(out_names),
                      check_rep=False),
            keep_unused=True)

    def __call__(self, in_maps):
        assert len(in_maps) == NCORES
        concat = [
            np.concatenate([np.asarray(m[n]) for m in in_maps], axis=0)
            for n in self._in_names
        ]
        concat += [
            np.zeros((NCORES * z.shape[0], *z.shape[1:]), z.dtype)
            for z in self._zero_outs
        ]
        out_arrs = self._fn(*concat)
        results = []
        for c in range(NCORES):
            d = {}
            for i, nm in enumerate(self._out_names):
                full = np.asarray(out_arrs[i])
                per = full.shape[0] // NCORES
                d[nm] = full[c * per:(c + 1) * per]
            results.append(d)
        return results


def _get_runners():
    if "ra" not in _cache:
        nc_a, nc_b = _get_kernels()
        _cache["ra"] = _Runner(nc_a)
        _cache["rb"] = _Runner(nc_b)
    return _cache["ra"], _cache["rb"]


def _fold_params(p):
    """Fold all network params into the device weight matrices (host, f64)."""
    Wout = p["Wout"].astype(np.float64)
    bout = p["bout"].astype(np.float64)
    attn_W = p["attn_W"].astype(np.float64)
    attn_b = p["attn_b"].astype(np.float64)
    W1 = Wout[0, :HID]          # fused part
    W2 = Wout[0, HID:2 * HID]   # t_Q part
    W3 = Wout[0, 2 * HID:]      # i_Q part

    # A_t[32h+d, h] = attn_W[h, d];  A_i[32h+d, h] = attn_W[h, 32+d]
    A_t = np.zeros((HID, H))
    A_i = np.zeros((HID, H))
    Bt = np.zeros((HID, H))
    for h in range(H):
        A_t[h * D:(h + 1) * D, h] = attn_W[h, :D]
        A_i[h * D:(h + 1) * D, h] = attn_W[h, D:]
        Bt[h * D:(h + 1) * D, h] = W1[h * D:(h + 1) * D]

    def WT(name):
        return p[name].astype(np.float64).T  # (IN, HID)

    wsmt = np.zeros((IN, 13))
    wsmt[:, 0:4] = WT("Wtq") @ A_t
    wsmt[:, 4:8] = WT("Wtv") @ Bt
    wsmt[:, 12] = WT("Wtq") @ W2
    wsmi = np.zeros((IN, 13))
    wsmi[:, 0:4] = WT("Wik") @ A_i
    wsmi[:, 8:12] = WT("Wiv") @ Bt
    wsmi[:, 12] = WT("Wiq") @ W3

    bsm = np.zeros(13)
    bsm[0:4] = (p["btq"].astype(np.float64) @ A_t
                + p["bik"].astype(np.float64) @ A_i + attn_b)
    bsm[4:8] = p["btv"].astype(np.float64) @ Bt
    bsm[8:12] = p["biv"].astype(np.float64) @ Bt
    bsm[12] = (p["btq"].astype(np.float64) @ W2
               + p["biq"].astype(np.float64) @ W3 + bout[0])

    thrt = f32(THRESH) - p["btv"].astype(f32)   # f32: matches device compare
    thri = f32(THRESH) - p["biv"].astype(f32)

    return {
        "wtv": _tf32(np.ascontiguousarray(WT("Wtv"), dtype=f32)),
        "wiv": _tf32(np.ascontiguousarray(WT("Wiv"), dtype=f32)),
        "wsmt": _tf32(wsmt.astype(f32)),
        "wsmi": _tf32(wsmi.astype(f32)),
        "thrt": thrt.reshape(HID, 1),
        "thri": thri.reshape(HID, 1),
        "bsm": bsm.astype(f32).reshape(13, 1),
        "ones": np.ones((1, 128), dtype=f32),
        "idt": np.eye(13, dtype=f32),
    }


def _chi_square_from_counts(S, C, L, B):
    """Replicate the reference chi-square given exact integer counts (f32 ops)."""
    F = S.shape[0]
    counts = np.zeros((F, 2, 2), dtype=f32)
    counts[:, 1, 1] = C
    counts[:, 1, 0] = S - C
    counts[:, 0, 1] = L - C
    counts[:, 0, 0] = B - S - L + C
    total = counts.sum(axis=(1, 2), dtype=f32)
    col = counts.sum(axis=1, dtype=f32)   # (F,2) over f_val -> label counts
    row = counts.sum(axis=2, dtype=f32)   # (F,2) over l_val -> feature counts
    expected = col[:, :, None] * row[:, None, :] / (total[:, None, None] + f32(1e-6))
    chi = ((counts - expected) ** 2 / (expected + f32(1e-6))).sum(
        axis=(1, 2), dtype=f32)
    return chi


def kernel(**inputs):
    text = _tf32(np.asarray(inputs["text_vec"], dtype=f32))
    image = _tf32(np.asarray(inputs["image_vec"], dtype=f32))
    label = np.asarray(inputs["label"]).astype(np.int64)

    folded = _fold_params(inputs)
    run_a, run_b = _get_runners()

    # Row assignment: sort all rows by label, deal contiguous RPC-row chunks
    # to cores, then within each core rotate the (at most one) mixed 512-row
    # block to device block index 31, so blocks 0..30 are label-pure and only
    # block 31 needs the on-device label-weighted count.
    order = np.concatenate([np.flatnonzero(label == 0),
                            np.flatnonzero(label != 0)])
    in_maps = []
    srcs = []
    pure1_masks = []
    for c in range(NCORES):
        chunk = order[c * RPC:(c + 1) * RPC]
        n0 = int((label[chunk] == 0).sum())
        k0, r0 = divmod(n0, BLK)
        if r0 > 0:
            src = np.concatenate([chunk[0:k0 * BLK], chunk[(k0 + 1) * BLK:],
                                  chunk[k0 * BLK:(k0 + 1) * BLK]])
        else:
            src = chunk
        lab_perm = (label[src] != 0)
        blocks = lab_perm.reshape(NBLK, BLK)
        pure1 = blocks.all(axis=1)
        mixed = blocks.any(axis=1) & ~pure1
        assert not mixed[:NBLK - 1].any(), "mixed block must be at index 31"
        m = {
            "xt": np.ascontiguousarray(text[src].T),
            "xi": np.ascontiguousarray(image[src].T),
            "lab": lab_perm[NBLK * BLK - BLK:].astype(f32).reshape(1, BLK),
        }
        m.update(folded)
        in_maps.append(m)
        srcs.append(src)
        pure1_masks.append(pure1[:NBLK - 1])

    # ---- launch A
    res_a = run_a(in_maps)

    # ---- host: reduce the tiny count tables, compute alpha (the "all-reduce")
    S_t = np.zeros(HID)
    S_i = np.zeros(HID)
    C_t = np.zeros(HID)
    C_i = np.zeros(HID)
    for c in range(NCORES):
        aux = res_a[c]["aux_out"].astype(np.float64)
        st = aux[:, 0:NBLK]
        si = aux[:, NBLK:2 * NBLK]
        S_t += st.sum(axis=1)
        S_i += si.sum(axis=1)
        p1 = pure1_masks[c]
        C_t += st[:, :NBLK - 1][:, p1].sum(axis=1) + aux[:, 2 * NBLK]
        C_i += si[:, :NBLK - 1][:, p1].sum(axis=1) + aux[:, 2 * NBLK + 1]
    L = float((label != 0).sum())
    chi_t = _chi_square_from_counts(S_t, C_t, L, float(B_TOT))
    chi_i = _chi_square_from_counts(S_i, C_i, L, float(B_TOT))
    chi_max = f32(max(chi_t.max(), chi_i.max()))
    alpha_t = (chi_t / (chi_max + f32(1e-6)))[:H].astype(f32)
    alpha_i = (chi_i / (chi_max + f32(1e-6)))[:H].astype(f32)

    coeffs = np.concatenate([alpha_t, alpha_i, -(alpha_t * alpha_i)]).astype(f32)
    crep = np.tile(np.tile(coeffs, 32)[None, :], (128, 1)).astype(f32)

    in_maps_b = [
        {"mb": np.hstack([res_a[c]["m_out"],
                          res_a[c]["aux_out"][:, 2 * NBLK + 2:]]),
         "crep": crep}
        for c in range(NCORES)
    ]

    # ---- launch B
    res_b = run_b(in_maps_b)

    # ---- gather (undo the per-core row permutation)
    out = np.empty((B_TOT, 1), dtype=f32)
    for c in range(NCORES):
        o = res_b[c]["o_out"]  # (128, NBLK*4); row r = col*128 + p
        rows = o.T.reshape(RPC)
        out[srcs[c], 0] = rows
    return out



# revision 28
# speedup vs baseline: 1.8025x; 1.8025x over previous
"""Trainium2 Bass kernel for nn_AttentionHAN (histogram_binning).

Strategy
--------
The reference network collapses algebraically (see baseline notes):
  - Per batch row the device needs 13 derived values:
      sp(4)  = pre-sigmoid attention scores
      tvd(4) = per-head dot of t_V with Wout[0,:128]
      ivd(4) = per-head dot of i_V with Wout[0,:128]
      base(1)= [t_Q, i_Q] @ Wout[0,128:] + bout
    plus chi-square count statistics of t_V/i_V: per-feature counts
    S = #(v > thr) and C = #(v > thr and label==1).
  - out[b] = base + sum_h [ at*m1 + ai*m2 - (at*ai)*m3 ],
      m1 = s*tvd, m2 = s*ivd, m3 = s*m2,  s = sigmoid(sp),
    where at/ai = alpha_t/alpha_i depend on the GLOBAL chi statistics.

This version (vs the fp32r baseline, 127us):
  - All device inputs/weights are float16 (PE matmul fp16 runs at the same
    1 cycle/column as fp32r but input DMA bytes halve: 32MB -> 16MB/core;
    end-to-end rel err vs the fp32 reference stays ~4e-4).
  - The 13-column "R" matmuls run transposed: lhsT = a 128-row batch slice
    of x, rhs = the folded (128,13) weight chunk, so each matmul streams
    only 13 output columns instead of 512 (208 vs 2048 PE cycles/block).
    A K=1 matmul (ones x bsm row) seeds the PSUM accumulator with the bias.
  - Launch A applies sigmoid (ACT) and forms m1/m2/m3 (GPSIMD) per row,
    so launch B is a single multiply (against a broadcast coefficient
    vector [at, ai, -at*ai, 1]) + segmented 13:1 reduce.
  - Work is spread so every engine stays under the 360GB/s DMA streaming
    rate (1456ns/512-row block): DVE does the two binarize+count ops,
    ACT does sigmoid + the r-value PSUM->SBUF copy, GPSIMD the m-products.
  - The one possibly-mixed label block is rotated to block 0 so its extra
    label-weighted count ops overlap the stream instead of extending the
    tail.

Sharding: pure data parallel over B on 8 cores (16384 rows each).  The tiny
per-core count tables are reduced on host (the "all-reduce" of the sharding
hint), alpha is computed exactly as the reference does, and launch B applies
the 13-coefficient combination per row.
"""

import sys
import numpy as np

sys.path.insert(0, "/opt/trn_rl_repo")

import concourse.bacc as bacc  # noqa: E402
import concourse.tile as tile  # noqa: E402
from concourse import mybir  # noqa: E402

F16 = mybir.dt.float16
F32 = mybir.dt.float32
f16 = np.float16
f32 = np.float32

B_TOT = 131072
IN = 256
HID = 128
H = 4
D = 32
NCORES = 8
THRESH = 0.7
BLK = 512
RPC = B_TOT // NCORES          # 16384 rows per core
NBLK = RPC // BLK              # 32 blocks of 512
SBCOLS = 2048                  # max superblock width (x-tile DMA granularity)
SUPER = [1024, 1024] + [2048] * 6 + [1024, 1024]  # superblock sizes
XBUFS = 4                      # x-tile buffering depth
PSUM_BUFS = 3                  # ptv/piv buffering depth
FLUSH = 8                      # blocks per R-staging flush
NPRIME = 14                    # PE-warmup pad matmuls
BCHUNK = 2                     # kernel B pipeline chunks

# cf16 packed-constant column offsets
_WTV0, _WTV1 = 0, 128
_WIV0, _WIV1 = 256, 384
_WSMT0, _WSMT1 = 512, 525
_WSMI0, _WSMI1 = 538, 551
_BSM = 564                     # bsm tiled x4 (52 cols)
_ONES = 616
_LAB = 744
_CF16 = 744 + BLK

_cache = {}


def _build_kernel_a():
    nc = bacc.Bacc("TRN2", target_bir_lowering=False, debug=False)
    xt = nc.dram_tensor("xt", (IN, RPC), F16, kind="ExternalInput")
    xi = nc.dram_tensor("xi", (IN, RPC), F16, kind="ExternalInput")
    cf16 = nc.dram_tensor("cf16", (128, _CF16), F16, kind="ExternalInput")
    cf32 = nc.dram_tensor("cf32", (128, 3), F32, kind="ExternalInput")
    r_out = nc.dram_tensor("r_out", (128, 52 * NBLK), F16,
                           kind="ExternalOutput")
    aux_out = nc.dram_tensor("aux_out", (128, 2 * NBLK + 2), F32,
                             kind="ExternalOutput")

    with tile.TileContext(nc) as tc:
        with (
            tc.tile_pool(name="w", bufs=1) as wp,
            tc.tile_pool(name="x", bufs=XBUFS) as xp,
            tc.tile_pool(name="fv", bufs=2) as fp,
            tc.tile_pool(name="acc", bufs=1) as ap,
            tc.tile_pool(name="rt", bufs=2) as rp,
            tc.tile_pool(name="ptv", bufs=3, space="PSUM") as ptvp,
            tc.tile_pool(name="piv", bufs=3, space="PSUM") as pivp,
            tc.tile_pool(name="pr", bufs=2, space="PSUM") as prp,
        ):
            cf = wp.tile([128, _CF16], F16, tag="cf16")
            cw = wp.tile([128, 3], F32, tag="cf32")
            nc.sync.dma_start(cf[:], cf16[:])
            nc.sync.dma_start(cw[:], cf32[:])
            wtv_sb = [cf[:, _WTV0:_WTV0 + 128], cf[:, _WTV1:_WTV1 + 128]]
            wiv_sb = [cf[:, _WIV0:_WIV0 + 128], cf[:, _WIV1:_WIV1 + 128]]
            wsmt_sb = [cf[:, _WSMT0:_WSMT0 + 13], cf[:, _WSMT1:_WSMT1 + 13]]
            wsmi_sb = [cf[:, _WSMI0:_WSMI0 + 13], cf[:, _WSMI1:_WSMI1 + 13]]
            bsm52_sb = cf[0:1, _BSM:_BSM + 52]
            ones_sb = cf[0:1, _ONES:_ONES + 128]
            lab_sb = cf[0:1, _LAB:_LAB + BLK]
            thrt_sb = cw[:, 0:1]
            thri_sb = cw[:, 1:2]
            sgni_sb = cw[:, 2:3]

            aux_sb = ap.tile([128, 2 * NBLK + 2], F32, tag="aux")

            # PE warm-up: dependency-free pads ramp the tensor engine to the
            # full p-state while the first x superblock streams in.
            for _ in range(NPRIME):
                pad = ptvp.tile([128, BLK], F32, name="pad", tag="ptv")
                nc.tensor.matmul(pad[:], ones_sb, lab_sb, start=True,
                                 stop=True)

            blk = 0
            rt = None
            for sb, size in enumerate(SUPER):
                off = blk * BLK
                xt0 = xp.tile([128, SBCOLS], F16, tag="xt0")
                xt1 = xp.tile([128, SBCOLS], F16, tag="xt1")
                xi0 = xp.tile([128, SBCOLS], F16, tag="xi0")
                xi1 = xp.tile([128, SBCOLS], F16, tag="xi1")
                nc.sync.dma_start(xt0[:, :size], xt[0:128, off:off + size])
                nc.sync.dma_start(xt1[:, :size], xt[128:256, off:off + size])
                nc.sync.dma_start(xi0[:, :size], xi[0:128, off:off + size])
                nc.sync.dma_start(xi1[:, :size], xi[128:256, off:off + size])
                for j in range(size // BLK):
                    o = j * BLK
                    if blk % FLUSH == 0:
                        rt = rp.tile([128, 52 * FLUSH], F16, tag="rt")
                    ptv = ptvp.tile([128, BLK], F32, name="ptv", tag="ptv")
                    piv = pivp.tile([128, BLK], F32, name="piv", tag="piv")
                    pr = prp.tile([128, 52], F32, name="pr", tag="pr")
                    nc.tensor.matmul(ptv[:], wtv_sb[0], xt0[:, o:o + BLK],
                                     start=True, stop=False)
                    nc.tensor.matmul(ptv[:], wtv_sb[1], xt1[:, o:o + BLK],
                                     start=False, stop=True)
                    nc.tensor.matmul(piv[:], wiv_sb[0], xi0[:, o:o + BLK],
                                     start=True, stop=False)
                    nc.tensor.matmul(piv[:], wiv_sb[1], xi1[:, o:o + BLK],
                                     start=False, stop=True)
                    nc.tensor.matmul(pr[:], ones_sb, bsm52_sb,
                                     start=True, stop=False,
                                     skip_group_check=True)
                    for c in range(4):
                        sl = pr[:, c * 13:(c + 1) * 13]
                        oc = o + c * 128
                        nc.tensor.matmul(sl, xt0[:, oc:oc + 128], wsmt_sb[0],
                                         start=False, stop=False,
                                         skip_group_check=True)
                        nc.tensor.matmul(sl, xt1[:, oc:oc + 128], wsmt_sb[1],
                                         start=False, stop=False,
                                         skip_group_check=True)
                        nc.tensor.matmul(sl, xi0[:, oc:oc + 128], wsmi_sb[0],
                                         start=False, stop=False,
                                         skip_group_check=True)
                        nc.tensor.matmul(sl, xi1[:, oc:oc + 128], wsmi_sb[1],
                                         start=False, stop=True,
                                         skip_group_check=True)
                    # binarize + S count for both modalities (DVE)
                    fvt = fp.tile([128, BLK], F16, tag="fvt")
                    nc.vector.tensor_scalar(
                        fvt[:], ptv[:], thrt_sb, None,
                        op0=mybir.AluOpType.is_gt, op1=mybir.AluOpType.add,
                        accum_out=aux_sb[:, blk:blk + 1])
                    fvi = fp.tile([128, BLK], F16, tag="fvi")
                    nc.vector.tensor_scalar(
                        fvi[:], piv[:], thri_sb, None,
                        op0=mybir.AluOpType.is_gt, op1=mybir.AluOpType.add,
                        accum_out=aux_sb[:, NBLK + blk:NBLK + blk + 1])
                    # R: s = sigmoid(sp) [ACT]; copy r4..r12 [ACT];
                    # m1/m2/m3 in place [GPSIMD]
                    pr3 = pr[:].rearrange("p (g k) -> p g k", k=13)
                    s = fp.tile([128, 16], F16, tag="s")
                    s3 = s[:].rearrange("p (g k) -> p g k", k=4)
                    nc.scalar.activation(s3, pr3[:, :, 0:4],
                                         mybir.ActivationFunctionType.Sigmoid)
                    rt3 = rt[:, (blk % FLUSH) * 52:(blk % FLUSH + 1) * 52] \
                        .rearrange("p (g k) -> p g k", k=13)
                    nc.scalar.activation(rt3[:, :, 4:13], pr3[:, :, 4:13],
                                         mybir.ActivationFunctionType.Copy)
                    nc.gpsimd.tensor_tensor(rt3[:, :, 0:4], s3,
                                            rt3[:, :, 4:8],
                                            op=mybir.AluOpType.mult)
                    nc.gpsimd.tensor_tensor(rt3[:, :, 4:8], s3,
                                            rt3[:, :, 8:12],
                                            op=mybir.AluOpType.mult)
                    nc.gpsimd.tensor_tensor(rt3[:, :, 8:12], s3,
                                            rt3[:, :, 4:8],
                                            op=mybir.AluOpType.mult)
                    if blk == 0:
                        # the single possibly-mixed block (rotated to the
                        # front): label-weighted counts.  Label row broadcast
                        # across partitions via a K=1 matmul.
                        plab = ptvp.tile([128, BLK], F32, name="plab",
                                         tag="ptv")
                        nc.tensor.matmul(plab[:], ones_sb, lab_sb,
                                         start=True, stop=True)
                        fvl = fp.tile([128, BLK], F16, tag="fvl")
                        nc.vector.scalar_tensor_tensor(
                            fvl[:], fvt[:], 1.0, plab[:],
                            op0=mybir.AluOpType.mult,
                            op1=mybir.AluOpType.mult,
                            accum_out=aux_sb[:, 2 * NBLK:2 * NBLK + 1])
                        nc.vector.scalar_tensor_tensor(
                            fvl[:], fvi[:], 1.0, plab[:],
                            op0=mybir.AluOpType.mult,
                            op1=mybir.AluOpType.mult,
                            accum_out=aux_sb[:, 2 * NBLK + 1:2 * NBLK + 2])
                    if blk % FLUSH == FLUSH - 1:
                        g = blk // FLUSH
                        nc.scalar.dma_start(
                            r_out[:, g * 52 * FLUSH:(g + 1) * 52 * FLUSH],
                            rt[:])
                    blk += 1
            nc.sync.dma_start(aux_out[:], aux_sb[:])

    nc.compile()
    return nc


def _build_kernel_b():
    from concourse.bass import broadcast_tensor_aps

    nc = bacc.Bacc("TRN2", target_bir_lowering=False, debug=False)
    rb = nc.dram_tensor("rb", (128, 52 * NBLK), F16, kind="ExternalInput")
    w13 = nc.dram_tensor("w13", (128, 13), F16, kind="ExternalInput")
    o_out = nc.dram_tensor("o_out", (128, 4 * NBLK), F16,
                           kind="ExternalOutput")

    nch = 4 * NBLK
    bounds = [0, 3 * nch // 4, nch]   # uneven: big chunk first, small last
    with tile.TileContext(nc) as tc:
        with tc.tile_pool(name="s", bufs=1) as sp:
            w13_sb = sp.tile([128, 13], F16, tag="w13")
            rb_sb = sp.tile([128, 52 * NBLK], F16, tag="rb")
            mm = sp.tile([128, 52 * NBLK], F16, tag="mm")
            out_sb = sp.tile([128, nch], F16, tag="o")
            w3 = w13_sb[:].rearrange("p (c k) -> p c k", k=13)
            for ch in range(len(bounds) - 1):
                c0, c1 = bounds[ch], bounds[ch + 1]
                nc.sync.dma_start(rb_sb[:, c0 * 13:c1 * 13],
                                  rb[:, c0 * 13:c1 * 13])
            nc.sync.dma_start(w13_sb[:], w13[:])
            for ch in range(len(bounds) - 1):
                c0, c1 = bounds[ch], bounds[ch + 1]
                rb3 = rb_sb[:, c0 * 13:c1 * 13].rearrange(
                    "p (c k) -> p c k", k=13)
                mm3 = mm[:, c0 * 13:c1 * 13].rearrange(
                    "p (c k) -> p c k", k=13)
                rb3b, w3b = broadcast_tensor_aps(rb3, w3)
                nc.vector.tensor_tensor(mm3, rb3b, w3b,
                                        op=mybir.AluOpType.mult)
                with nc.allow_low_precision(reason="13-term fp16 row reduce"):
                    nc.vector.tensor_reduce(
                        out_sb[:, c0:c1], mm3,
                        axis=mybir.AxisListType.X, op=mybir.AluOpType.add)
                nc.sync.dma_start(o_out[:, c0:c1], out_sb[:, c0:c1])

    nc.compile()
    return nc


def _get_kernels():
    if "a" not in _cache:
        _cache["a"] = _build_kernel_a()
        _cache["b"] = _build_kernel_b()
    return _cache["a"], _cache["b"]


class _Runner:
    """Persistent jitted SPMD executor for a compiled Bass module.

    Mirrors bass2jax.run_bass_via_pjrt but keeps the jitted callable alive so
    repeated kernel() invocations skip retracing/recompilation."""

    def __init__(self, nc):
        import jax
        from jax.sharding import Mesh, PartitionSpec
        from jax.experimental.shard_map import shard_map
        from concourse import bass2jax

        bass2jax.install_neuronx_cc_hook()
        self._nc = nc
        pname = nc.partition_id_tensor.name if nc.partition_id_tensor else None
        in_names, out_names, out_avals = [], [], []
        self._zero_outs = []
        for alloc in nc.m.functions[0].allocations:
            if not isinstance(alloc, mybir.MemoryLocationSet):
                continue
            nm = alloc.memorylocations[0].name
            if alloc.kind == "ExternalInput":
                if nm != pname:
                    in_names.append(nm)
            elif alloc.kind == "ExternalOutput":
                out_names.append(nm)
                shape = tuple(alloc.tensor_shape)
                dt = mybir.dt.np(alloc.dtype)
                out_avals.append(jax.core.ShapedArray(shape, dt))
                self._zero_outs.append(np.zeros(shape, dt))
        self._in_names = in_names
        self._out_names = out_names
        all_in_names = in_names + out_names + ([pname] if pname else [])

        def _body(*args):
            operands = list(args)
            if pname:
                operands.append(bass2jax.partition_id_tensor())
            outs = bass2jax._bass_exec_p.bind(
                *operands, out_avals=tuple(out_avals),
                in_names=tuple(all_in_names), out_names=tuple(out_names),
                lowering_input_output_aliases=(), sim_require_finite=True,
                sim_require_nnan=True, nc=nc)
            return tuple(outs)

        devices = jax.devices()[:NCORES]
        assert len(devices) == NCORES, f"need {NCORES} devices"
        mesh = Mesh(np.asarray(devices), ("core",))
        nio = len(in_names) + len(out_names)
        self._fn = jax.jit(
            shard_map(_body, mesh=mesh,
                      in_specs=(PartitionSpec("core"),) * nio,
                      out_specs=(PartitionSpec("core"),) * len(out_names),
                      check_rep=False),
            keep_unused=True)

    def __call__(self, in_maps):
        assert len(in_maps) == NCORES
        concat = [
            np.concatenate([np.asarray(m[n]) for m in in_maps], axis=0)
            for n in self._in_names
        ]
        concat += [
            np.zeros((NCORES * z.shape[0], *z.shape[1:]), z.dtype)
            for z in self._zero_outs
        ]
        out_arrs = self._fn(*concat)
        results = []
        for c in range(NCORES):
            d = {}
            for i, nm in enumerate(self._out_names):
                full = np.asarray(out_arrs[i])
                per = full.shape[0] // NCORES
                d[nm] = full[c * per:(c + 1) * per]
            results.append(d)
        return results


def _get_runners():
    if "ra" not in _cache:
        nc_a, nc_b = _get_kernels()
        _cache["ra"] = _Runner(nc_a)
        _cache["rb"] = _Runner(nc_b)
    return _cache["ra"], _cache["rb"]


def _fold_params(p):
    """Fold all network params into the device weight matrices (host, f64)."""
    Wout = p["Wout"].astype(np.float64)
    bout = p["bout"].astype(np.float64)
    attn_W = p["attn_W"].astype(np.float64)
    attn_b = p["attn_b"].astype(np.float64)
    W1 = Wout[0, :HID]          # fused part
    W2 = Wout[0, HID:2 * HID]   # t_Q part
    W3 = Wout[0, 2 * HID:]      # i_Q part

    # A_t[32h+d, h] = attn_W[h, d];  A_i[32h+d, h] = attn_W[h, 32+d]
    A_t = np.zeros((HID, H))
    A_i = np.zeros((HID, H))
    Bt = np.zeros((HID, H))
    for h in range(H):
        A_t[h * D:(h + 1) * D, h] = attn_W[h, :D]
        A_i[h * D:(h + 1) * D, h] = attn_W[h, D:]
        Bt[h * D:(h + 1) * D, h] = W1[h * D:(h + 1) * D]

    def WT(name):
        return p[name].astype(np.float64).T  # (IN, HID)

    wsmt = np.zeros((IN, 13))
    wsmt[:, 0:4] = WT("Wtq") @ A_t
    wsmt[:, 4:8] = WT("Wtv") @ Bt
    wsmt[:, 12] = WT("Wtq") @ W2
    wsmi = np.zeros((IN, 13))
    wsmi[:, 0:4] = WT("Wik") @ A_i
    wsmi[:, 8:12] = WT("Wiv") @ Bt
    wsmi[:, 12] = WT("Wiq") @ W3

    bsm = np.zeros(13)
    bsm[0:4] = (p["btq"].astype(np.float64) @ A_t
                + p["bik"].astype(np.float64) @ A_i + attn_b)
    bsm[4:8] = p["btv"].astype(np.float64) @ Bt
    bsm[8:12] = p["biv"].astype(np.float64) @ Bt
    bsm[12] = (p["btq"].astype(np.float64) @ W2
               + p["biq"].astype(np.float64) @ W3 + bout[0])

    cf32 = np.zeros((128, 3), dtype=f32)
    cf32[:, 0] = f32(THRESH) - p["btv"].astype(f32)   # t threshold
    cf32[:, 1] = f32(THRESH) - p["biv"].astype(f32)   # i threshold
    cf32[:, 2] = p["biv"].astype(f32) - f32(THRESH)   # i Sign bias
    return {
        "wtv": WT("Wtv").astype(f16),     # (256, 128)
        "wiv": WT("Wiv").astype(f16),
        "wsmt": wsmt.astype(f16),         # (256, 13)
        "wsmi": wsmi.astype(f16),
        "bsm": bsm.astype(f16),           # (13,)
        "cf32": cf32,
    }


def _build_cf16(folded, lab_row):
    cf = np.zeros((128, _CF16), dtype=f16)
    cf[:, _WTV0:_WTV0 + 128] = folded["wtv"][0:128]
    cf[:, _WTV1:_WTV1 + 128] = folded["wtv"][128:256]
    cf[:, _WIV0:_WIV0 + 128] = folded["wiv"][0:128]
    cf[:, _WIV1:_WIV1 + 128] = folded["wiv"][128:256]
    cf[:, _WSMT0:_WSMT0 + 13] = folded["wsmt"][0:128]
    cf[:, _WSMT1:_WSMT1 + 13] = folded["wsmt"][128:256]
    cf[:, _WSMI0:_WSMI0 + 13] = folded["wsmi"][0:128]
    cf[:, _WSMI1:_WSMI1 + 13] = folded["wsmi"][128:256]
    cf[0, _BSM:_BSM + 52] = np.tile(folded["bsm"], 4)
    cf[0, _ONES:_ONES + 128] = f16(1.0)
    cf[0, _LAB:_LAB + BLK] = lab_row
    return cf


def _chi_square_from_counts(S, C, L, B):
    """Replicate the reference chi-square given exact integer counts (f32 ops)."""
    F = S.shape[0]
    counts = np.zeros((F, 2, 2), dtype=f32)
    counts[:, 1, 1] = C
    counts[:, 1, 0] = S - C
    counts[:, 0, 1] = L - C
    counts[:, 0, 0] = B - S - L + C
    total = counts.sum(axis=(1, 2), dtype=f32)
    col = counts.sum(axis=1, dtype=f32)   # (F,2) over f_val -> label counts
    row = counts.sum(axis=2, dtype=f32)   # (F,2) over l_val -> feature counts
    expected = col[:, :, None] * row[:, None, :] / (total[:, None, None] + f32(1e-6))
    chi = ((counts - expected) ** 2 / (expected + f32(1e-6))).sum(
        axis=(1, 2), dtype=f32)
    return chi


def kernel(**inputs):
    text = np.asarray(inputs["text_vec"], dtype=f32).astype(f16)
    image = np.asarray(inputs["image_vec"], dtype=f32).astype(f16)
    label = np.asarray(inputs["label"]).astype(np.int64)

    folded = _fold_params(inputs)
    run_a, run_b = _get_runners()

    # Row assignment: sort all rows by label, deal contiguous RPC-row chunks
    # to cores, then within each core rotate the (at most one) mixed 512-row
    # block to device block index 0, so blocks 1..31 are label-pure and only
    # block 0 needs the on-device label-weighted count (overlapped with the
    # stream instead of extending the tail).
    order = np.concatenate([np.flatnonzero(label == 0),
                            np.flatnonzero(label != 0)])
    in_maps = []
    srcs = []
    pure1_masks = []
    l0s = []
    for c in range(NCORES):
        chunk = order[c * RPC:(c + 1) * RPC]
        n0 = int((label[chunk] == 0).sum())
        k0, r0 = divmod(n0, BLK)
        if r0 > 0:
            src = np.concatenate([chunk[k0 * BLK:(k0 + 1) * BLK],
                                  chunk[0:k0 * BLK],
                                  chunk[(k0 + 1) * BLK:]])
        else:
            src = chunk
        lab_perm = (label[src] != 0)
        blocks = lab_perm.reshape(NBLK, BLK)
        pure1 = blocks.all(axis=1)
        mixed = blocks.any(axis=1) & ~pure1
        assert not mixed[1:].any(), "mixed block must be at index 0"
        m = {
            "xt": np.ascontiguousarray(text[src].T),
            "xi": np.ascontiguousarray(image[src].T),
            "cf16": _build_cf16(folded, lab_perm[:BLK].astype(f16)),
            "cf32": folded["cf32"],
        }
        in_maps.append(m)
        srcs.append(src)
        pure1_masks.append(pure1[1:])
        l0s.append(float(lab_perm[:BLK].sum()))

    # ---- launch A
    res_a = run_a(in_maps)

    # ---- host: reduce the tiny count tables, compute alpha (the "all-reduce")
    S_t = np.zeros(HID)
    S_i = np.zeros(HID)
    C_t = np.zeros(HID)
    C_i = np.zeros(HID)
    for c in range(NCORES):
        aux = res_a[c]["aux_out"].astype(np.float64)
        st = aux[:, 0:NBLK]
        si = aux[:, NBLK:2 * NBLK]
        S_t += st.sum(axis=1)
        S_i += si.sum(axis=1)
        p1 = pure1_masks[c]
        C_t += st[:, 1:][:, p1].sum(axis=1) + aux[:, 2 * NBLK]
        C_i += si[:, 1:][:, p1].sum(axis=1) + aux[:, 2 * NBLK + 1]
    L = float((label != 0).sum())
    chi_t = _chi_square_from_counts(S_t, C_t, L, float(B_TOT))
    chi_i = _chi_square_from_counts(S_i, C_i, L, float(B_TOT))
    chi_max = f32(max(chi_t.max(), chi_i.max()))
    alpha_t = (chi_t / (chi_max + f32(1e-6)))[:H].astype(f32)
    alpha_i = (chi_i / (chi_max + f32(1e-6)))[:H].astype(f32)

    w13 = np.concatenate([alpha_t, alpha_i, -(alpha_t * alpha_i),
                          [f32(1.0)]]).astype(f16)
    w13_t = np.ascontiguousarray(
        np.broadcast_to(w13[None, :], (128, 13)))

    in_maps_b = [{"rb": res_a[c]["r_out"], "w13": w13_t}
                 for c in range(NCORES)]

    # ---- launch B
    res_b = run_b(in_maps_b)

    # ---- gather (undo the per-core row permutation)
    out = np.empty((B_TOT, 1), dtype=f32)
    for c in range(NCORES):
        o = res_b[c]["o_out"].astype(f32)  # (128, 128); row = col*128 + p
        rows = o.T.reshape(RPC)
        out[srcs[c], 0] = rows
    return out
